# revision 38
# baseline (speedup 1.0000x reference)
"""Trainium2 Bass kernel for nn_HC2STARModel (partitioned-norm + center/domain MLPs).

Strategy:
  - Host sorts rows by domain; 2 cores per domain (8 cores, 4 domains), so each
    core runs ONE domain's MLP. Feature-major activations: x ships as 2*x fp8,
    per-tile contiguous [128, 8*S]; weights ship as 32*W fp8 blobs.
  - All big matmuls are fp8 DoubleRow (2 K-chunks per instruction): stats sums
    (ones stationary, M=16), L1 (+mean correction as an extra DR pair), L2, L3.
    The PE is LDWEIGHTS/issue-limited (~216ns per DR matmul at N=512), so
    instruction count is minimized; moving free dim is PSUM-capped at 512.
  - LayerNorm: mean/var/rsqrt chain runs on the DVE with a bit-trick Newton
    rsqrt (no ACT abs_reciprocal_sqrt => all ACT functions fit ONE activation
    table set, sigmoid_and_others, pinned by a dummy Sigmoid; zero mid-kernel
    ACT_TABLE_LOADs). eps is dropped (1e-5 vs var~1) and 1 Newton step
    suffices (~2e-3 on invstd, output rel err ~4e-4 vs 2e-2 budget).
  - invstd is applied at L2 eviction (a per-sample scalar commutes with the
    feature contraction), so L1 evicts on ACT (Relu*scale, fp8 out) and only
    4 DVE scalar_tensor_tensor evictions per tile remain. x^2 is computed on
    ACT (Square, fp8 out) because DVE fp8-out tensor ops run at 1x rate.
  - Software pipeline per round: front_a(t+1) [xt DMA, sum, mean] ->
    L1(t) -> front_b(t+1) [square, sumsq, rsqrt chain, GpSimd partition
    broadcast] -> L2(t) -> epilogue(t-1) [L3, tanh-fuse, head, sigmoid, out].
    Tiles: remainder first (cheap pipeline prime), then full 512s.
    Weight DMAs are issued before anything else on three queues (w1 split
    3 ways) because the prologue is HBM-bandwidth-bound and w1 arrival gates
    the first L1 and the PE HAM clock-gate warmup.
  - Tail: the body PE stream is 99% busy, so the exit is the only slack. The
    last TWO tiles' epilogues each split into 256-wide halves running on
    disjoint PSUM rings (borrowing the then-dead stats/L1/L2 pool buffers via
    their existing tags -- no extra banks), giving four overlapping narrow
    chains instead of two serial wide ones: tail shrinks ~12us -> ~7us.
  - b1 == 0 and b2 == 0 are required (true for this model) and asserted.
"""
import os
import sys

sys.path.insert(0, "/opt/trn_rl_repo")

import numpy as np
import ml_dtypes

BF16 = ml_dtypes.bfloat16
FP8 = ml_dtypes.float8_e4m3

B, D_IN = 16384, 1024
N_DOM = 4
H1, H2, H3, FH = 512, 256, 128, 64
EPS = 1e-5
P = 128
NT = 512  # batch-tile (moving free dim) size
MAGIC = 0x5F3759DF
_cache = {}
LAST_RESULTS = None  # stash for test harness profiling


def _sizes_for(S):
    sizes = []
    off = 0
    rem = S % NT
    if rem:
        sizes.append((0, rem))
        off = rem
    while off < S:
        n = min(NT, S - off)
        sizes.append((off, n))
        off += n
    return sizes


def _build(S):
    from concourse import bass, bacc, tile
    import concourse.mybir as mybir

    dt = mybir.dt
    AF = mybir.ActivationFunctionType
    Alu = mybir.AluOpType
    DR = mybir.MatmulPerfMode.DoubleRow
    DRS = mybir.MatmulPerfMode.DoubleRow

    sizes = _sizes_for(S)

    nc = bacc.Bacc("TRN2", target_bir_lowering=False, debug=False)

    xT = nc.declare_dram_parameter("xT", [P, 8 * S], dt.float8e4, isOutput=False)
    w1 = nc.declare_dram_parameter("w1", [P, 8, 8, P], dt.float8e4, isOutput=False)
    w2 = nc.declare_dram_parameter("w2", [P, 8, H2], dt.float8e4, isOutput=False)
    w3 = nc.declare_dram_parameter("w3", [P, 4, P], dt.float8e4, isOutput=False)
    fwb = nc.declare_dram_parameter("fwb", [P, FH + 1], dt.bfloat16, isOutput=False)
    brow8 = nc.declare_dram_parameter("brow8", [1, 8, 2, P], dt.float8e4,
                                      isOutput=False)
    bcols = nc.declare_dram_parameter("bcols", [P, 8], dt.float32, isOutput=False)
    out = nc.declare_dram_parameter("out", [1, S], dt.float32, isOutput=True)

    with tile.TileContext(nc) as tc:
        with (
            tc.tile_pool(name="wp", bufs=1) as wp,
            tc.tile_pool(name="cst", bufs=1) as cst,
            tc.tile_pool(name="xp", bufs=3) as xp,
            tc.tile_pool(name="ap", bufs=3) as ap,
            tc.tile_pool(name="ps_st", bufs=1, space=bass.MemorySpace.PSUM) as ps_st,
            tc.tile_pool(name="ps_sq", bufs=1, space=bass.MemorySpace.PSUM) as ps_sq,
            tc.tile_pool(name="ps_l1", bufs=2, space=bass.MemorySpace.PSUM) as ps_l1,
            tc.tile_pool(name="ps_l2", bufs=2, space=bass.MemorySpace.PSUM) as ps_l2,
            tc.tile_pool(name="ps_ep", bufs=1, space=bass.MemorySpace.PSUM) as ps_ep,
            tc.tile_pool(name="ps_hd", bufs=1, space=bass.MemorySpace.PSUM) as ps_hd,
        ):
            # ALL DMA configs first: nothing (memsets, table loads) may delay
            # the weight transfers, whose arrival gates the first L1 and the
            # HAM clock-gate warmup
            xt0 = xp.tile([P, 8, sizes[0][1]], dt.float8e4, tag="xt")
            nc.sync.dma_start(out=xt0[:], in_=xT[:, 8 * sizes[0][0]:
                                                 8 * (sizes[0][0] + sizes[0][1])])
            w1_sb = wp.tile([P, 8, 8, P], dt.float8e4, tag="w1")
            nc.scalar.dma_start(out=w1_sb[:, 0:3, :, :], in_=w1[:, 0:3, :, :])
            nc.gpsimd.dma_start(out=w1_sb[:, 3:6, :, :], in_=w1[:, 3:6, :, :])
            nc.sync.dma_start(out=w1_sb[:, 6:8, :, :], in_=w1[:, 6:8, :, :])
            brow8_sb = wp.tile([1, 8, 2, P], dt.float8e4, tag="brow8")
            nc.gpsimd.dma_start(out=brow8_sb[:], in_=brow8[:])
            w2_sb = wp.tile([P, 8, H2], dt.float8e4, tag="w2")
            nc.scalar.dma_start(out=w2_sb[:], in_=w2[:])
            bcols_sb = wp.tile([P, 8], dt.float32, tag="bcols")
            nc.gpsimd.dma_start(out=bcols_sb[:], in_=bcols[:])
            w3_sb = wp.tile([P, 4, P], dt.float8e4, tag="w3")
            nc.gpsimd.dma_start(out=w3_sb[:], in_=w3[:])
            fwb_sb = wp.tile([P, FH + 1], dt.bfloat16, tag="fwb")
            nc.gpsimd.dma_start(out=fwb_sb[:], in_=fwb[:])

            ones8 = cst.tile([P, 2, 16], dt.float8e4, tag="ones8")
            nc.vector.memset(ones8[:], 1.0)
            magicrow = cst.tile([1, NT], dt.int32, tag="magicrow")
            nc.vector.memset(magicrow[:], MAGIC)
            dum = cst.tile([1, 1], dt.float32, tag="dum")
            nc.vector.memset(dum[:], 0.0)
            # dummy Sigmoid pins the ACT table set to sigmoid_and_others
            # (Square/Relu/Tanh/Sigmoid all live there -> no reloads)
            nc.scalar.activation(dum[:], dum[:], AF.Sigmoid)

            def front_a(col, N, xt=None):
                """xt DMA + sum reduction + mean row (the part L1 needs)."""
                if xt is None:
                    xt = xp.tile([P, 8, N], dt.float8e4, tag="xt")
                    nc.sync.dma_start(out=xt[:], in_=xT[:, 8 * col:8 * (col + N)])
                st = ps_st.tile([16, N], dt.float32, tag="st")
                for c in range(4):
                    nc.tensor.matmul(st[0:16, :], ones8[:], xt[:, 2 * c:2 * c + 2, :],
                                     start=(c == 0), stop=(c == 3), perf_mode=DR)
                # st_sum = 2048*mu; m2 = 64*mu (f32, SBUF); mean4 rows = 2*mu fp8
                m2 = ap.tile([1, N], dt.float32, tag="m2")
                nc.vector.tensor_scalar(m2[:], st[0:1, :], 1.0 / 32.0, None,
                                        Alu.mult)
                mean4 = ap.tile([1, 2, N], dt.float8e4, tag="mean4")
                nc.vector.tensor_scalar(mean4[0:1, 0, :], st[0:1, :],
                                        1.0 / 1024.0, None, Alu.mult)
                nc.vector.tensor_scalar(mean4[0:1, 1, :], st[0:1, :],
                                        1.0 / 1024.0, None, Alu.mult)
                return [col, N, xt, mean4, m2, None, st]

            def front_b(state):
                """Square + sumsq + var/rsqrt chain (GpSimd) + broadcast.
                Runs late in the round: inv64 is only needed by the NEXT
                round's L2 evictions."""
                col, N, xt, mean4, m2, _, st = state
                xsq = xp.tile([P, 8, N], dt.float8e4, tag="xsq")
                nc.scalar.activation(xsq[:], xt[:], AF.Square)
                stq = ps_sq.tile([16, N], dt.float32, tag="stq")
                for c in range(4):
                    nc.tensor.matmul(stq[0:16, :], ones8[:], xsq[:, 2 * c:2 * c + 2, :],
                                     start=(c == 0), stop=(c == 3), perf_mode=DR)
                # stq = 4096*E[x^2]; sq0 f32 copy to SBUF
                sq0 = ap.tile([1, N], dt.float32, tag="sq0")
                nc.vector.tensor_scalar(sq0[:], stq[0:1, :], 1.0, None, Alu.mult)
                # GpSimd: v = sq0 - m2^2 = 4096*var; rsqrt via magic + 1 Newton
                msq = ap.tile([1, N], dt.float32, tag="msq")
                nc.vector.tensor_mul(msq[:], m2[:], m2[:])
                v = ap.tile([1, N], dt.float32, tag="v")
                nc.vector.tensor_sub(v[:], sq0[:], msq[:])
                s1 = ap.tile([1, N], dt.int32, tag="s1")
                nc.vector.tensor_scalar(s1[:], v[:].bitcast(dt.int32), 1, None,
                                        Alu.arith_shift_right)
                s2 = ap.tile([1, N], dt.int32, tag="s2")
                nc.vector.tensor_tensor(s2[:], magicrow[0:1, 0:N], s1[:],
                                        Alu.subtract)
                y0 = s2[:].bitcast(dt.float32)
                u = ap.tile([1, N], dt.float32, tag="u")
                nc.vector.tensor_mul(u[:], y0, y0)
                w_ = ap.tile([1, N], dt.float32, tag="w_")
                nc.vector.scalar_tensor_tensor(w_[:], v[:], -0.5, u[:],
                                               Alu.mult, Alu.mult)
                invrow = ap.tile([1, N], dt.float32, tag="invrow")
                nc.vector.scalar_tensor_tensor(invrow[:], w_[:], 1.5, y0,
                                               Alu.add, Alu.mult)
                inv64 = ap.tile([P, N], dt.float32, tag="inv64")
                nc.gpsimd.partition_broadcast(inv64[:], invrow[:])
                state[5] = inv64

            front_cur = front_a(*sizes[0], xt=xt0)

            def mid_l1(state):
                # L1: out-chunks o=0..3 center, 4..7 domain; all fp8 DR;
                # mean correction rides a DR pair; eviction on ACT
                col, N, xt, mean4, m2, inv64 = state[:6]
                h1 = ap.tile([P, 8, N], dt.float8e4, tag="h1")
                for o in range(8):
                    p1 = ps_l1.tile([P, N], dt.float32, tag="p1")
                    if N > P:
                        for c in range(4):
                            nc.tensor.matmul(p1[:],
                                             w1_sb[:, o, 2 * c:2 * c + 2, :],
                                             xt[:, 2 * c:2 * c + 2, :],
                                             start=(c == 0), stop=False,
                                             perf_mode=DRS)
                    else:
                        # remainder tile: at FD<=128 DoubleRow loses (256-col
                        # LDWEIGHTS, no FWL); normal-mode matmuls let FWL cut
                        # the weight loads ~4x and keep the PE array busier
                        # (earlier HAM warm) during the DMA-bound prologue
                        for c in range(8):
                            nc.tensor.matmul(p1[:], w1_sb[:, o, c, :],
                                             xt[:, c, :],
                                             start=(c == 0), stop=False)
                    nc.tensor.matmul(p1[:], brow8_sb[0:1, o, :, :], mean4[:],
                                     start=False, stop=True, perf_mode=DRS)
                    nc.scalar.activation(h1[:, o, :], p1[:], AF.Relu, scale=0.5)
                return h1

            def mid_l2(state, h1):
                # L2 center/domain: fp8 DR; eviction applies invstd on DVE
                col, N, xt, mean4, m2, inv64 = state[:6]
                h2 = ap.tile([P, 4, N], dt.float8e4, tag="h2")
                for (base, hoff) in ((0, 0), (4, 2)):
                    for o in range(2):
                        p2 = ps_l2.tile([P, N], dt.float32, tag="p2")
                        for c in range(2):
                            nc.tensor.matmul(
                                p2[:],
                                w2_sb[:, base + 2 * c:base + 2 * c + 2,
                                      o * P:(o + 1) * P],
                                h1[:, base + 2 * c:base + 2 * c + 2, :],
                                start=(c == 0), stop=(c == 1), perf_mode=DRS)
                        nc.vector.scalar_tensor_tensor(h2[:, hoff + o, :], p2[:],
                                                       0.0, inv64[:],
                                                       Alu.max, Alu.mult)
                return (col, N, h2)

            def ep_stage(state, c0=None, c1=None, pp=None, hp=None):
                col, N, h2 = state
                if c0 is None:
                    c0, c1 = 0, N
                n = c1 - c0
                ppool, ptag = pp if pp else (ps_ep, "p3")
                hpool, htag = hp if hp else (ps_hd, "ph")
                # L3 domain -> tanh (bias on ACT); L3 center fused into hf
                p3d = ppool.tile([P, n], dt.float32, tag=ptag)
                nc.tensor.matmul(p3d[:], w3_sb[:, 2:4, :], h2[:, 2:4, c0:c1],
                                 start=True, stop=True, perf_mode=DRS)
                t3 = ap.tile([P, n], dt.bfloat16, tag="t3")
                nc.scalar.activation(t3[:], p3d[:], AF.Tanh, scale=1.0 / 512.0,
                                     bias=bcols_sb[:, 5:6])
                p3c = ppool.tile([P, n], dt.float32, tag=ptag)
                nc.tensor.matmul(p3c[:], w3_sb[:, 0:2, :], h2[:, 0:2, c0:c1],
                                 start=True, stop=True, perf_mode=DRS)
                hf = ap.tile([P, n], dt.bfloat16, tag="hf")
                nc.vector.scalar_tensor_tensor(hf[:], p3c[:], bcols_sb[:, 4:5],
                                               t3[:], Alu.add, Alu.mult)

                # head: 128 -> 64 (relu) -> 1 -> sigmoid
                ph = hpool.tile([FH, n], dt.float32, tag=htag)
                nc.tensor.matmul(ph[:], fwb_sb[:, 0:FH], hf[:], start=True,
                                 stop=True)
                fh = ap.tile([FH, n], dt.bfloat16, tag="fh")
                nc.vector.tensor_scalar(fh[:], ph[:], bcols_sb[0:FH, 6:7],
                                        0.0, Alu.add, Alu.max)
                pm = hpool.tile([1, n], dt.float32, tag=htag)
                nc.tensor.matmul(pm[0:1, :], fwb_sb[0:FH, FH:FH + 1], fh[:],
                                 start=True, stop=True)
                orow = ap.tile([1, n], dt.float32, tag="orow")
                nc.scalar.activation(orow[:], pm[0:1, :], AF.Sigmoid,
                                     bias=bcols_sb[0:1, 7:8])
                nc.sync.dma_start(out=out[0:1, col + c0:col + c1], in_=orow[:])

            # tile0's var-chain runs in the prologue so inv64(0) is ready
            front_b(front_cur)

            prev = None
            for ti, (col, N) in enumerate(sizes):
                cur = front_cur
                # next tile's sum-front goes FIRST in every engine's stream;
                # its var-front (square/sumsq/chain) goes mid-round, after
                # this tile's L1, because inv64 is only needed next round
                if ti + 1 < len(sizes):
                    front_cur = front_a(*sizes[ti + 1])
                h1 = mid_l1(cur)
                if ti + 1 < len(sizes):
                    front_b(front_cur)
                state = mid_l2(cur, h1)
                # previous tile's epilogue emits AFTER this tile's L1/L2 so
                # its ACT/DVE chains never stall the PE stream. The LAST
                # in-loop ep starts the tail critical path: split it into
                # halves on independent PSUM rings so the chains overlap.
                if prev is not None:
                    if ti == len(sizes) - 1 and prev[1] > 2 * P:
                        hh = (prev[1] // 2 + P - 1) // P * P
                        ep_stage(prev, 0, hh)
                        ep_stage(prev, hh, prev[1], pp=(ps_l2, "p2"),
                                 hp=(ps_st, "st"))
                    else:
                        ep_stage(prev)
                prev = state
            # final epilogue is the un-overlapped exit path: split it into
            # halves so the two serial chains pipeline against each other.
            # The second half borrows the (now dead) L1/L2 PSUM rings so the
            # two chains never serialize on the ps_ep/ps_hd single buffers.
            if prev[1] > 2 * P:
                half = (prev[1] // 2 + P - 1) // P * P
                # first half on the dead L1/stats rings; second half back
                # on ep/hd (free again once the split ep above drains)
                ep_stage(prev, 0, half, pp=(ps_l1, "p1"), hp=(ps_sq, "stq"))
                ep_stage(prev, half, prev[1])
            else:
                ep_stage(prev)

    nc.compile()
    return nc


def _prep_core(x_rows, dmn, prm, S):
    """Build the per-core input map for one core handling domain `dmn`."""
    cW1 = prm["cW1"]
    dW1, db1 = prm["dW1"][dmn], prm["db1"][dmn]
    pnw, pnb = prm["pn_w"][dmn], prm["pn_b"][dmn]

    W1cat_raw = np.concatenate([cW1, dW1], axis=1)           # (1024, 1024)
    W1cat = W1cat_raw * pnw[:, None]
    b1 = np.concatenate([prm["cb1"], db1]) + pnb @ W1cat_raw  # (1024,)
    assert float(np.max(np.abs(b1))) == 0.0, "kernel requires b1 == 0"
    assert float(np.max(np.abs(prm["cb2"]))) == 0.0, "kernel requires cb2 == 0"
    assert float(np.max(np.abs(prm["db2"][dmn]))) == 0.0, "requires db2 == 0"

    de = prm["dom_emb"][dmn]
    aux = np.maximum(de @ prm["aW1"] + prm["ab1"], 0.0) @ prm["aW2"] + prm["ab2"]

    # weights ship as fp8 e4m3 at 32x; x ships as 2*x. Scale ledger:
    #   p1 = (32W)(2x) = 64*z1 (+DR correction -64*colsum*mu)
    #   h1 = Relu(p1)/2 = 32*relu(z1)                    [ACT, fp8]
    #   p2 = (32W2)(32relu z1) = 1024*y2; h2 = max(p2,0)*inv/64 = 16*relu(z2)
    #   p3 = (32W3)(16relu z2) = 512*z3; t3 = tanh(p3/512 + b3d)
    #   hf = (p3c + 512*cb3)*t3 = 512*h_fused; fw1 pre-divided by 512
    w1q = np.clip(32.0 * W1cat, -240, 240).astype(FP8)
    colsum1q = w1q.astype(np.float32).sum(axis=0) / 32.0

    # w1 SBUF layout: [p][o][k][m]
    w1o = np.ascontiguousarray(
        w1q.astype(np.float32).reshape(8, P, 8, P).transpose(2, 1, 0, 3)).astype(FP8)

    brow8 = np.zeros((1, 8, 2, P), np.float32)
    for o in range(8):
        brow8[0, o, 0, :] = -16.0 * colsum1q[o * P:(o + 1) * P]
        brow8[0, o, 1, :] = -16.0 * colsum1q[o * P:(o + 1) * P]
    brow8v = np.clip(brow8, -240, 240).astype(FP8)

    def shp8(w, nchunk):  # (K, M) -> (128, K//128, M) fp8 SBUF layout at 32x
        return np.ascontiguousarray(np.clip(32.0 * w, -240, 240)
                                    .reshape(nchunk, P, w.shape[1])
                                    .transpose(1, 0, 2)).astype(FP8)

    w2cat = np.concatenate([shp8(prm["cW2"], 4), shp8(prm["dW2"][dmn], 4)],
                           axis=1)                            # (128, 8, 256)
    w3cat = np.concatenate([shp8(prm["cW3"], 2), shp8(prm["dW3"][dmn], 2)],
                           axis=1)                            # (128, 4, 128)

    fwb = np.zeros((P, FH + 1), np.float32)
    fwb[:, 0:FH] = prm["fW1"] / 512.0
    fwb[0:FH, FH] = prm["fW2"][:, 0]

    bcols = np.zeros((P, 8), np.float32)
    bcols[:, 4] = 512.0 * prm["cb3"]
    bcols[:, 5] = prm["db3"][dmn]
    bcols[:FH, 6] = prm["fb1"]
    bcols[0, 7] = prm["fb2"][0] + aux[0]

    # x: per-tile contiguous fp8 blob [128, 8*S]; tile (off,n) occupies
    # byte cols 8*off .. 8*(off+n), laid out as [chunk][col] per partition
    xc = np.zeros((S, D_IN), np.float32)
    xc[: len(x_rows)] = x_rows
    x8 = np.clip(2.0 * xc, -240, 240).astype(FP8)             # (S, 1024)
    xk = np.ascontiguousarray(x8.T.reshape(8, P, S).transpose(1, 0, 2))  # (P,8,S)
    blob = np.empty((P, 8 * S), FP8)
    for (off, n) in _sizes_for(S):
        seg = xk[:, :, off:off + n].reshape(P, 8 * n)
        blob[:, 8 * off:8 * (off + n)] = seg

    return {
        "xT": blob,
        "w1": w1o,
        "w2": w2cat,
        "w3": w3cat,
        "fwb": fwb.astype(BF16),
        "brow8": brow8v,
        "bcols": bcols,
    }


def kernel(**inputs):
    global LAST_RESULTS
    from concourse.bass_utils import run_bass_kernel_spmd

    prm = {k: np.asarray(v, np.float32) for k, v in inputs.items()
           if k not in ("domain_ids",)}
    x = prm["x"]
    dom = np.asarray(inputs["domain_ids"]).astype(np.int64).reshape(-1)
    in_dtype = np.asarray(inputs["x"]).dtype

    order = np.argsort(dom, kind="stable")
    sorted_dom = dom[order]
    bounds = np.searchsorted(sorted_dom, np.arange(N_DOM + 1))
    core_rows, core_dom = [], []
    for d in range(N_DOM):
        idx = order[bounds[d]:bounds[d + 1]]
        h = (len(idx) + 1) // 2
        core_rows += [idx[:h], idx[h:]]
        core_dom += [d, d]

    S = max(len(r) for r in core_rows)
    S = max(((S + P - 1) // P) * P, P)

    in_maps = [_prep_core(x[core_rows[c]], core_dom[c], prm, S)
               for c in range(8)]

    if S not in _cache:
        _cache[S] = _build(S)
    nc = _cache[S]

    trace = bool(int(os.environ.get("KERNEL_TRACE", "0")))
    try:
        res = run_bass_kernel_spmd(nc, in_maps, list(range(8)), trace=trace)
    except Exception:
        # transient device hiccups (NRT_EXEC_UNIT_UNRECOVERABLE etc.) clear
        # on retry
        res = run_bass_kernel_spmd(nc, in_maps, list(range(8)), trace=trace)
    LAST_RESULTS = res

    out = np.zeros((B, 1), np.float32)
    for c in range(8):
        o = np.asarray(res.results[c]["out"], np.float32).reshape(-1)
        out[core_rows[c], 0] = o[: len(core_rows[c])]
    return out.astype(in_dtype)


# revision 39
# speedup vs baseline: 1.0112x; 1.0112x over previous
"""Trainium2 Bass kernel for nn_HC2STARModel (partitioned-norm + center/domain MLPs).

Strategy:
  - Host sorts rows by domain; 2 cores per domain (8 cores, 4 domains), so each
    core runs ONE domain's MLP. Feature-major activations: x ships as 2*x fp8,
    per-tile contiguous [128, 8*S]; weights ship as 32*W fp8 blobs.
  - All big matmuls are fp8 DoubleRow (2 K-chunks per instruction): stats sums
    (ones stationary, M=16), L1 (+mean correction as an extra DR pair), L2, L3.
    The PE is LDWEIGHTS/issue-limited (~216ns per DR matmul at N=512), so
    instruction count is minimized; moving free dim is PSUM-capped at 512.
  - LayerNorm: mean/var/rsqrt chain runs on the DVE with a bit-trick Newton
    rsqrt (no ACT abs_reciprocal_sqrt => all ACT functions fit ONE activation
    table set, sigmoid_and_others, pinned by a dummy Sigmoid; zero mid-kernel
    ACT_TABLE_LOADs). eps is dropped (1e-5 vs var~1) and 1 Newton step
    suffices (~2e-3 on invstd, output rel err ~4e-4 vs 2e-2 budget).
  - invstd is applied at L2 eviction (a per-sample scalar commutes with the
    feature contraction), so L1 evicts on ACT (Relu*scale, fp8 out) and only
    4 DVE scalar_tensor_tensor evictions per tile remain. x^2 is computed on
    ACT (Square, fp8 out) because DVE fp8-out tensor ops run at 1x rate.
  - Software pipeline per round: front_a(t+1) [xt DMA, sum, mean] ->
    L1(t) -> front_b(t+1) [square, sumsq, rsqrt chain, GpSimd partition
    broadcast] -> L2(t) -> epilogue(t-1) [L3, tanh-fuse, head, sigmoid, out].
    Tiles: remainder first (cheap pipeline prime), then full 512s.
    Weight DMAs are issued before anything else on three queues (w1 split
    3 ways) because the prologue is HBM-bandwidth-bound and w1 arrival gates
    the first L1 and the PE HAM clock-gate warmup.
  - Tail: the body PE stream is 99% busy, so the exit is the only slack. The
    last TWO tiles' epilogues each split into 256-wide halves running on
    disjoint PSUM rings (borrowing the then-dead stats/L1/L2 pool buffers via
    their existing tags -- no extra banks), giving four overlapping narrow
    chains instead of two serial wide ones: tail shrinks ~12us -> ~7us.
  - b1 == 0 and b2 == 0 are required (true for this model) and asserted.
"""
import os
import sys

sys.path.insert(0, "/opt/trn_rl_repo")

import numpy as np
import ml_dtypes

BF16 = ml_dtypes.bfloat16
FP8 = ml_dtypes.float8_e4m3

B, D_IN = 16384, 1024
N_DOM = 4
H1, H2, H3, FH = 512, 256, 128, 64
EPS = 1e-5
P = 128
NT = 512  # batch-tile (moving free dim) size
MAGIC = 0x5F3759DF
_cache = {}
LAST_RESULTS = None  # stash for test harness profiling


def _sizes_for(S):
    sizes = []
    off = 0
    rem = S % NT
    if rem:
        sizes.append((0, rem))
        off = rem
    while off < S:
        n = min(NT, S - off)
        sizes.append((off, n))
        off += n
    return sizes


def _build(S):
    from concourse import bass, bacc, tile
    import concourse.mybir as mybir

    dt = mybir.dt
    AF = mybir.ActivationFunctionType
    Alu = mybir.AluOpType
    DR = mybir.MatmulPerfMode.DoubleRow
    DRS = mybir.MatmulPerfMode.DoubleRow

    sizes = _sizes_for(S)

    nc = bacc.Bacc("TRN2", target_bir_lowering=False, debug=False)

    xT = nc.declare_dram_parameter("xT", [P, 8 * S], dt.float8e4, isOutput=False)
    w1 = nc.declare_dram_parameter("w1", [P, 8, 8, P], dt.float8e4, isOutput=False)
    w2 = nc.declare_dram_parameter("w2", [P, 8, H2], dt.float8e4, isOutput=False)
    w3 = nc.declare_dram_parameter("w3", [P, 4, P], dt.float8e4, isOutput=False)
    fwb = nc.declare_dram_parameter("fwb", [P, FH + 1], dt.bfloat16, isOutput=False)
    brow8 = nc.declare_dram_parameter("brow8", [1, 8, 2, P], dt.float8e4,
                                      isOutput=False)
    bcols = nc.declare_dram_parameter("bcols", [P, 8], dt.float32, isOutput=False)
    out = nc.declare_dram_parameter("out", [1, S], dt.float32, isOutput=True)

    with tile.TileContext(nc) as tc:
        with (
            tc.tile_pool(name="wp", bufs=1) as wp,
            tc.tile_pool(name="cst", bufs=1) as cst,
            tc.tile_pool(name="xp", bufs=3) as xp,
            tc.tile_pool(name="ap", bufs=3) as ap,
            tc.tile_pool(name="ps_st", bufs=1, space=bass.MemorySpace.PSUM) as ps_st,
            tc.tile_pool(name="ps_sq", bufs=1, space=bass.MemorySpace.PSUM) as ps_sq,
            tc.tile_pool(name="ps_l1", bufs=2, space=bass.MemorySpace.PSUM) as ps_l1,
            tc.tile_pool(name="ps_l2", bufs=2, space=bass.MemorySpace.PSUM) as ps_l2,
            tc.tile_pool(name="ps_ep", bufs=1, space=bass.MemorySpace.PSUM) as ps_ep,
            tc.tile_pool(name="ps_hd", bufs=1, space=bass.MemorySpace.PSUM) as ps_hd,
        ):
            # ALL DMA configs first: nothing (memsets, table loads) may delay
            # the weight transfers, whose arrival gates the first L1 and the
            # HAM clock-gate warmup
            xt0 = xp.tile([P, 8, sizes[0][1]], dt.float8e4, tag="xt")
            nc.sync.dma_start(out=xt0[:], in_=xT[:, 8 * sizes[0][0]:
                                                 8 * (sizes[0][0] + sizes[0][1])])
            w1_sb = wp.tile([P, 8, 8, P], dt.float8e4, tag="w1")
            nc.scalar.dma_start(out=w1_sb[:, 0:3, :, :], in_=w1[:, 0:3, :, :])
            nc.gpsimd.dma_start(out=w1_sb[:, 3:6, :, :], in_=w1[:, 3:6, :, :])
            nc.sync.dma_start(out=w1_sb[:, 6:8, :, :], in_=w1[:, 6:8, :, :])
            brow8_sb = wp.tile([1, 8, 2, P], dt.float8e4, tag="brow8")
            nc.gpsimd.dma_start(out=brow8_sb[:], in_=brow8[:])
            w2_sb = wp.tile([P, 8, H2], dt.float8e4, tag="w2")
            nc.scalar.dma_start(out=w2_sb[:], in_=w2[:])
            bcols_sb = wp.tile([P, 8], dt.float32, tag="bcols")
            nc.gpsimd.dma_start(out=bcols_sb[:], in_=bcols[:])
            w3_sb = wp.tile([P, 4, P], dt.float8e4, tag="w3")
            nc.gpsimd.dma_start(out=w3_sb[:], in_=w3[:])
            fwb_sb = wp.tile([P, FH + 1], dt.bfloat16, tag="fwb")
            nc.gpsimd.dma_start(out=fwb_sb[:], in_=fwb[:])

            ones8 = cst.tile([P, 2, 16], dt.float8e4, tag="ones8")
            nc.vector.memset(ones8[:], 1.0)
            magicrow = cst.tile([1, NT], dt.int32, tag="magicrow")
            nc.vector.memset(magicrow[:], MAGIC)
            dum = cst.tile([1, 1], dt.float32, tag="dum")
            nc.vector.memset(dum[:], 0.0)
            # dummy Sigmoid pins the ACT table set to sigmoid_and_others
            # (Square/Relu/Tanh/Sigmoid all live there -> no reloads)
            nc.scalar.activation(dum[:], dum[:], AF.Sigmoid)

            def front_a(col, N, xt=None):
                """xt DMA + sum reduction + mean row (the part L1 needs)."""
                if xt is None:
                    xt = xp.tile([P, 8, N], dt.float8e4, tag="xt")
                    nc.sync.dma_start(out=xt[:], in_=xT[:, 8 * col:8 * (col + N)])
                st = ps_st.tile([16, N], dt.float32, tag="st")
                for c in range(4):
                    nc.tensor.matmul(st[0:16, :], ones8[:], xt[:, 2 * c:2 * c + 2, :],
                                     start=(c == 0), stop=(c == 3), perf_mode=DR)
                # st_sum = 2048*mu; m2 = 64*mu (f32, SBUF); mean4 rows = 2*mu fp8
                m2 = ap.tile([1, N], dt.float32, tag="m2")
                nc.vector.tensor_scalar(m2[:], st[0:1, :], 1.0 / 32.0, None,
                                        Alu.mult)
                mean4 = ap.tile([1, 2, N], dt.float8e4, tag="mean4")
                nc.vector.tensor_scalar(mean4[0:1, 0, :], st[0:1, :],
                                        1.0 / 1024.0, None, Alu.mult)
                nc.vector.tensor_scalar(mean4[0:1, 1, :], st[0:1, :],
                                        1.0 / 1024.0, None, Alu.mult)
                return [col, N, xt, mean4, m2, None, st]

            def front_b(state):
                """Square + sumsq + var/rsqrt chain (GpSimd) + broadcast.
                Runs late in the round: inv64 is only needed by the NEXT
                round's L2 evictions."""
                col, N, xt, mean4, m2, _, st = state
                xsq = xp.tile([P, 8, N], dt.float8e4, tag="xsq")
                nc.scalar.activation(xsq[:], xt[:], AF.Square)
                stq = ps_sq.tile([16, N], dt.float32, tag="stq")
                for c in range(4):
                    nc.tensor.matmul(stq[0:16, :], ones8[:], xsq[:, 2 * c:2 * c + 2, :],
                                     start=(c == 0), stop=(c == 3), perf_mode=DR)
                # stq = 4096*E[x^2]; sq0 f32 copy to SBUF
                sq0 = ap.tile([1, N], dt.float32, tag="sq0")
                nc.vector.tensor_scalar(sq0[:], stq[0:1, :], 1.0, None, Alu.mult)
                # GpSimd: v = sq0 - m2^2 = 4096*var; rsqrt via magic + 1 Newton
                msq = ap.tile([1, N], dt.float32, tag="msq")
                nc.vector.tensor_mul(msq[:], m2[:], m2[:])
                v = ap.tile([1, N], dt.float32, tag="v")
                nc.vector.tensor_sub(v[:], sq0[:], msq[:])
                s1 = ap.tile([1, N], dt.int32, tag="s1")
                nc.vector.tensor_scalar(s1[:], v[:].bitcast(dt.int32), 1, None,
                                        Alu.arith_shift_right)
                s2 = ap.tile([1, N], dt.int32, tag="s2")
                nc.vector.tensor_tensor(s2[:], magicrow[0:1, 0:N], s1[:],
                                        Alu.subtract)
                y0 = s2[:].bitcast(dt.float32)
                u = ap.tile([1, N], dt.float32, tag="u")
                nc.vector.tensor_mul(u[:], y0, y0)
                w_ = ap.tile([1, N], dt.float32, tag="w_")
                nc.vector.scalar_tensor_tensor(w_[:], v[:], -0.5, u[:],
                                               Alu.mult, Alu.mult)
                invrow = ap.tile([1, N], dt.float32, tag="invrow")
                nc.vector.scalar_tensor_tensor(invrow[:], w_[:], 1.5, y0,
                                               Alu.add, Alu.mult)
                inv64 = ap.tile([P, N], dt.float32, tag="inv64")
                nc.gpsimd.partition_broadcast(inv64[:], invrow[:])
                state[5] = inv64

            front_cur = front_a(*sizes[0], xt=xt0)

            def mid_l1(state):
                # L1: out-chunks o=0..3 center, 4..7 domain; all fp8 DR;
                # mean correction rides a DR pair; eviction on ACT
                col, N, xt, mean4, m2, inv64 = state[:6]
                h1 = ap.tile([P, 8, N], dt.float8e4, tag="h1")
                for o in range(8):
                    p1 = ps_l1.tile([P, N], dt.float32, tag="p1")
                    for c in range(4):
                        nc.tensor.matmul(p1[:], w1_sb[:, o, 2 * c:2 * c + 2, :],
                                         xt[:, 2 * c:2 * c + 2, :],
                                         start=(c == 0), stop=False, perf_mode=DRS)
                    nc.tensor.matmul(p1[:], brow8_sb[0:1, o, :, :], mean4[:],
                                     start=False, stop=True, perf_mode=DRS)
                    nc.scalar.activation(h1[:, o, :], p1[:], AF.Relu, scale=0.5)
                return h1

            def mid_l2(state, h1):
                # L2 center/domain: fp8 DR; eviction applies invstd on DVE
                col, N, xt, mean4, m2, inv64 = state[:6]
                h2 = ap.tile([P, 4, N], dt.float8e4, tag="h2")
                for (base, hoff) in ((0, 0), (4, 2)):
                    for o in range(2):
                        p2 = ps_l2.tile([P, N], dt.float32, tag="p2")
                        for c in range(2):
                            nc.tensor.matmul(
                                p2[:],
                                w2_sb[:, base + 2 * c:base + 2 * c + 2,
                                      o * P:(o + 1) * P],
                                h1[:, base + 2 * c:base + 2 * c + 2, :],
                                start=(c == 0), stop=(c == 1), perf_mode=DRS)
                        nc.vector.scalar_tensor_tensor(h2[:, hoff + o, :], p2[:],
                                                       0.0, inv64[:],
                                                       Alu.max, Alu.mult)
                return (col, N, h2)

            def ep_stage(state, c0=None, c1=None, pp=None, hp=None):
                col, N, h2 = state
                if c0 is None:
                    c0, c1 = 0, N
                n = c1 - c0
                ppool, ptag = pp if pp else (ps_ep, "p3")
                hpool, htag = hp if hp else (ps_hd, "ph")
                # L3 domain -> tanh (bias on ACT); L3 center fused into hf
                p3d = ppool.tile([P, n], dt.float32, tag=ptag)
                nc.tensor.matmul(p3d[:], w3_sb[:, 2:4, :], h2[:, 2:4, c0:c1],
                                 start=True, stop=True, perf_mode=DRS)
                t3 = ap.tile([P, n], dt.bfloat16, tag="t3")
                nc.scalar.activation(t3[:], p3d[:], AF.Tanh, scale=1.0 / 512.0,
                                     bias=bcols_sb[:, 5:6])
                p3c = ppool.tile([P, n], dt.float32, tag=ptag)
                nc.tensor.matmul(p3c[:], w3_sb[:, 0:2, :], h2[:, 0:2, c0:c1],
                                 start=True, stop=True, perf_mode=DRS)
                hf = ap.tile([P, n], dt.bfloat16, tag="hf")
                nc.vector.scalar_tensor_tensor(hf[:], p3c[:], bcols_sb[:, 4:5],
                                               t3[:], Alu.add, Alu.mult)

                # head: 128 -> 64 (relu) -> 1 -> sigmoid
                ph = hpool.tile([FH, n], dt.float32, tag=htag)
                nc.tensor.matmul(ph[:], fwb_sb[:, 0:FH], hf[:], start=True,
                                 stop=True)
                fh = ap.tile([FH, n], dt.bfloat16, tag="fh")
                nc.vector.tensor_scalar(fh[:], ph[:], bcols_sb[0:FH, 6:7],
                                        0.0, Alu.add, Alu.max)
                pm = hpool.tile([1, n], dt.float32, tag=htag)
                nc.tensor.matmul(pm[0:1, :], fwb_sb[0:FH, FH:FH + 1], fh[:],
                                 start=True, stop=True)
                orow = ap.tile([1, n], dt.float32, tag="orow")
                nc.scalar.activation(orow[:], pm[0:1, :], AF.Sigmoid,
                                     bias=bcols_sb[0:1, 7:8])
                nc.sync.dma_start(out=out[0:1, col + c0:col + c1], in_=orow[:])

            # tile0's var-chain runs in the prologue so inv64(0) is ready
            front_b(front_cur)

            prev = None
            for ti, (col, N) in enumerate(sizes):
                cur = front_cur
                # next tile's sum-front goes FIRST in every engine's stream;
                # its var-front (square/sumsq/chain) goes mid-round, after
                # this tile's L1, because inv64 is only needed next round
                if ti + 1 < len(sizes):
                    front_cur = front_a(*sizes[ti + 1])
                h1 = mid_l1(cur)
                if ti + 1 < len(sizes):
                    front_b(front_cur)
                state = mid_l2(cur, h1)
                # previous tile's epilogue emits AFTER this tile's L1/L2 so
                # its ACT/DVE chains never stall the PE stream. The LAST
                # in-loop ep starts the tail critical path: split it into
                # halves on independent PSUM rings so the chains overlap.
                if prev is not None:
                    if ti == len(sizes) - 1 and prev[1] > 2 * P:
                        hh = (prev[1] // 2 + P - 1) // P * P
                        ep_stage(prev, 0, hh)
                        ep_stage(prev, hh, prev[1], pp=(ps_l2, "p2"),
                                 hp=(ps_st, "st"))
                    else:
                        ep_stage(prev)
                prev = state
            # final epilogue is the un-overlapped exit path: split it into
            # halves so the two serial chains pipeline against each other.
            # The second half borrows the (now dead) L1/L2 PSUM rings so the
            # two chains never serialize on the ps_ep/ps_hd single buffers.
            if prev[1] > 2 * P:
                half = (prev[1] // 2 + P - 1) // P * P
                # first half on the dead L1/stats rings; second half back
                # on ep/hd (free again once the split ep above drains)
                ep_stage(prev, 0, half, pp=(ps_l1, "p1"), hp=(ps_sq, "stq"))
                ep_stage(prev, half, prev[1])
            else:
                ep_stage(prev)

    nc.compile()
    return nc


def _prep_core(x_rows, dmn, prm, S):
    """Build the per-core input map for one core handling domain `dmn`."""
    cW1 = prm["cW1"]
    dW1, db1 = prm["dW1"][dmn], prm["db1"][dmn]
    pnw, pnb = prm["pn_w"][dmn], prm["pn_b"][dmn]

    W1cat_raw = np.concatenate([cW1, dW1], axis=1)           # (1024, 1024)
    W1cat = W1cat_raw * pnw[:, None]
    b1 = np.concatenate([prm["cb1"], db1]) + pnb @ W1cat_raw  # (1024,)
    assert float(np.max(np.abs(b1))) == 0.0, "kernel requires b1 == 0"
    assert float(np.max(np.abs(prm["cb2"]))) == 0.0, "kernel requires cb2 == 0"
    assert float(np.max(np.abs(prm["db2"][dmn]))) == 0.0, "requires db2 == 0"

    de = prm["dom_emb"][dmn]
    aux = np.maximum(de @ prm["aW1"] + prm["ab1"], 0.0) @ prm["aW2"] + prm["ab2"]

    # weights ship as fp8 e4m3 at 32x; x ships as 2*x. Scale ledger:
    #   p1 = (32W)(2x) = 64*z1 (+DR correction -64*colsum*mu)
    #   h1 = Relu(p1)/2 = 32*relu(z1)                    [ACT, fp8]
    #   p2 = (32W2)(32relu z1) = 1024*y2; h2 = max(p2,0)*inv/64 = 16*relu(z2)
    #   p3 = (32W3)(16relu z2) = 512*z3; t3 = tanh(p3/512 + b3d)
    #   hf = (p3c + 512*cb3)*t3 = 512*h_fused; fw1 pre-divided by 512
    w1q = np.clip(32.0 * W1cat, -240, 240).astype(FP8)
    colsum1q = w1q.astype(np.float32).sum(axis=0) / 32.0

    # w1 SBUF layout: [p][o][k][m]
    w1o = np.ascontiguousarray(
        w1q.astype(np.float32).reshape(8, P, 8, P).transpose(2, 1, 0, 3)).astype(FP8)

    brow8 = np.zeros((1, 8, 2, P), np.float32)
    for o in range(8):
        brow8[0, o, 0, :] = -16.0 * colsum1q[o * P:(o + 1) * P]
        brow8[0, o, 1, :] = -16.0 * colsum1q[o * P:(o + 1) * P]
    brow8v = np.clip(brow8, -240, 240).astype(FP8)

    def shp8(w, nchunk):  # (K, M) -> (128, K//128, M) fp8 SBUF layout at 32x
        return np.ascontiguousarray(np.clip(32.0 * w, -240, 240)
                                    .reshape(nchunk, P, w.shape[1])
                                    .transpose(1, 0, 2)).astype(FP8)

    w2cat = np.concatenate([shp8(prm["cW2"], 4), shp8(prm["dW2"][dmn], 4)],
                           axis=1)                            # (128, 8, 256)
    w3cat = np.concatenate([shp8(prm["cW3"], 2), shp8(prm["dW3"][dmn], 2)],
                           axis=1)                            # (128, 4, 128)

    fwb = np.zeros((P, FH + 1), np.float32)
    fwb[:, 0:FH] = prm["fW1"] / 512.0
    fwb[0:FH, FH] = prm["fW2"][:, 0]

    bcols = np.zeros((P, 8), np.float32)
    bcols[:, 4] = 512.0 * prm["cb3"]
    bcols[:, 5] = prm["db3"][dmn]
    bcols[:FH, 6] = prm["fb1"]
    bcols[0, 7] = prm["fb2"][0] + aux[0]

    # x: per-tile contiguous fp8 blob [128, 8*S]; tile (off,n) occupies
    # byte cols 8*off .. 8*(off+n), laid out as [chunk][col] per partition
    xc = np.zeros((S, D_IN), np.float32)
    xc[: len(x_rows)] = x_rows
    x8 = np.clip(2.0 * xc, -240, 240).astype(FP8)             # (S, 1024)
    xk = np.ascontiguousarray(x8.T.reshape(8, P, S).transpose(1, 0, 2))  # (P,8,S)
    blob = np.empty((P, 8 * S), FP8)
    for (off, n) in _sizes_for(S):
        seg = xk[:, :, off:off + n].reshape(P, 8 * n)
        blob[:, 8 * off:8 * (off + n)] = seg

    return {
        "xT": blob,
        "w1": w1o,
        "w2": w2cat,
        "w3": w3cat,
        "fwb": fwb.astype(BF16),
        "brow8": brow8v,
        "bcols": bcols,
    }


def kernel(**inputs):
    global LAST_RESULTS
    from concourse.bass_utils import run_bass_kernel_spmd

    prm = {k: np.asarray(v, np.float32) for k, v in inputs.items()
           if k not in ("domain_ids",)}
    x = prm["x"]
    dom = np.asarray(inputs["domain_ids"]).astype(np.int64).reshape(-1)
    in_dtype = np.asarray(inputs["x"]).dtype

    order = np.argsort(dom, kind="stable")
    sorted_dom = dom[order]
    bounds = np.searchsorted(sorted_dom, np.arange(N_DOM + 1))
    core_rows, core_dom = [], []
    for d in range(N_DOM):
        idx = order[bounds[d]:bounds[d + 1]]
        h = (len(idx) + 1) // 2
        core_rows += [idx[:h], idx[h:]]
        core_dom += [d, d]

    S = max(len(r) for r in core_rows)
    S = max(((S + P - 1) // P) * P, P)

    in_maps = [_prep_core(x[core_rows[c]], core_dom[c], prm, S)
               for c in range(8)]

    if S not in _cache:
        _cache[S] = _build(S)
    nc = _cache[S]

    trace = bool(int(os.environ.get("KERNEL_TRACE", "0")))
    try:
        res = run_bass_kernel_spmd(nc, in_maps, list(range(8)), trace=trace)
    except Exception:
        # transient device hiccups (NRT_EXEC_UNIT_UNRECOVERABLE etc.) clear
        # on retry
        res = run_bass_kernel_spmd(nc, in_maps, list(range(8)), trace=trace)
    LAST_RESULTS = res

    out = np.zeros((B, 1), np.float32)
    for c in range(8):
        o = np.asarray(res.results[c]["out"], np.float32).reshape(-1)
        out[core_rows[c], 0] = o[: len(core_rows[c])]
    return out.astype(in_dtype)
